# revision 1
# baseline (speedup 1.0000x reference)
"""Trainium2 Bass kernel for nn_GCNNet (3-layer GCN, 50k nodes, 800k edges,
HID=128, 64 graphs) sharded across 8 NeuronCores.

Strategy
--------
- Nodes are assigned to cores (load-balanced by in-degree), and within a core
  bin-packed into 98 windows of 64 "slot" columns each (6272 slots/core).
- GCN norm dinv[src]*dinv[dst] is factored: the src side is folded into the
  per-node feature table, the dst side is applied at PSUM eviction. This makes
  the scatter (segment-sum) selection matrices exact 0/1.
- Per layer, each core computes table rows hw' = dinv * (h @ W) for its nodes,
  split into a bf16 hi|lo pair (full fp32-grade precision), written as 512B
  rows, then AllGathered into a full 50176-row table.
- Edge messages are fetched with dma_gather (512B rows). int16 index limit is
  handled by splitting the table at row 25088 (= cores 0-3 vs 4-7) and issuing
  two gather calls per chunk; windows are packed so each half fits a fixed
  number of 128-edge K-tiles.
- Segment-sum runs on the TensorEngine: messages (hi then lo) are the FWL
  stationary operand, the 0/1 S-tile [128 edges x 64 dsts] is the moving
  operand, accumulating aggT[HID, dst] in PSUM across the window's tiles.
- Final graph mean-pool: z = h3 @ W_reg per node on-device, per-graph partial
  sums via a 0/1 pool matmul; host sums across cores, divides by counts, adds
  b_reg.
"""
import os
import sys
import numpy as np
import ml_dtypes

# ---- problem constants (hardcoded; kernel.py must be self-contained) ----
N = 50000
E = 800000
D_IN = 100
HID = 128
NL = 3
NG = 64

NCORES = 8
P = 128
W_DST = 64             # dst columns per window
NWIN = 98              # windows per core
NPCS = NWIN * W_DST    # 6272 slots per core
NGRP = NPCS // P       # 49 groups of 128 slots
T_HALF = 5             # K-tiles per src-half per window
T_WIN = 2 * T_HALF     # 10 tiles per window
NTILES = NWIN * T_WIN  # 980 tiles per core
NROWS = NCORES * NPCS  # 50176 table rows
HALF_ROW = NROWS // 2  # 25088 = cores 0..3
CPW = 7                # windows per gather/scatter chunk
NCHUNK = NWIN // CPW   # 14 chunks
DELEM = 2 * HID        # 256 bf16 per table row (hi|lo) = 512B

_cache = {}


# ======================= host preprocessing =======================

def _preprocess(edge_index, batch):
    src = np.asarray(edge_index[0], dtype=np.int64)
    dst = np.asarray(edge_index[1], dtype=np.int64)
    batch = np.asarray(batch, dtype=np.int64)

    deg = (np.bincount(dst, minlength=N) + 1).astype(np.float32)
    dinv = (1.0 / np.sqrt(deg)).astype(np.float32)

    # ---- node -> core assignment, balanced by in-degree(+1) ----
    w = deg.astype(np.int64)  # edges contributed as dst (incl self-loop)
    order = np.argsort(-w, kind="stable")
    core_load = np.zeros(NCORES, dtype=np.int64)
    core_cnt = np.zeros(NCORES, dtype=np.int64)
    node_core = np.empty(N, dtype=np.int32)
    NPC_REAL = N // NCORES  # 6250 real nodes per core
    for v in order:
        # pick feasible core with min load
        feas = np.nonzero(core_cnt < NPC_REAL)[0]
        c = feas[np.argmin(core_load[feas])]
        node_core[v] = c
        core_load[c] += w[v]
        core_cnt[c] += 1

    # half A = src owned by cores 0..3
    src_half = (node_core[src] >= 4).astype(np.int64)  # 0 = A, 1 = B

    # per-node (a, b) in-edge counts (self-loop counted by own core half)
    a_cnt = np.bincount(dst[src_half == 0], minlength=N)
    b_cnt = np.bincount(dst[src_half == 1], minlength=N)
    own_half = (node_core >= 4)
    a_cnt = a_cnt + (~own_half)
    b_cnt = b_cnt + own_half

    # ---- per-core window bin-packing ----
    CAP = T_HALF * P  # 640 per half
    node_slot = np.full(N, -1, dtype=np.int64)   # global slot = core*NPCS + local
    slot_node = np.full(NCORES * NPCS, -1, dtype=np.int64)
    for c in range(NCORES):
        nodes = np.nonzero(node_core == c)[0]
        tot = a_cnt[nodes] + b_cnt[nodes]
        nodes = nodes[np.argsort(-tot, kind="stable")]
        wa = np.zeros(NWIN, dtype=np.int64)
        wb = np.zeros(NWIN, dtype=np.int64)
        wc = np.zeros(NWIN, dtype=np.int64)
        for v in nodes:
            av, bv = a_cnt[v], b_cnt[v]
            feas = np.nonzero((wc < W_DST) & (wa + av <= CAP) & (wb + bv <= CAP))[0]
            if len(feas) == 0:
                raise RuntimeError("window packing overflow; raise T_HALF")
            # minimize resulting max(half) load, tie-break on count
            cost = np.maximum(wa[feas] + av, wb[feas] + bv) * 100 + wc[feas]
            j = feas[np.argmin(cost)]
            slot = c * NPCS + j * W_DST + wc[j]
            node_slot[v] = slot
            slot_node[slot] = v
            wa[j] += av
            wb[j] += bv
            wc[j] += 1

    perm = node_slot  # global table row of node v

    # ---- per-core edge slot assignment ----
    # edges (incl self-loops) grouped by (core, window, half); order within
    # a half arbitrary; k-th edge -> tile h*T_HALF + k//128, partition k%128
    all_src = np.concatenate([src, np.arange(N, dtype=np.int64)])
    all_dst = np.concatenate([dst, np.arange(N, dtype=np.int64)])
    e_row = perm[all_src]                 # table row of src
    e_half = (e_row >= HALF_ROW).astype(np.int64)
    e_rel = e_row - e_half * HALF_ROW     # row within half (int16-safe)
    e_dslot = perm[all_dst]               # dst slot (global)
    e_core = e_dslot // NPCS
    e_local = e_dslot % NPCS
    e_win = e_local // W_DST
    e_j = e_local % W_DST                 # dst column within window

    # sort edges by (core, win, half) for grouped fill
    key = ((e_core * NWIN + e_win) * 2 + e_half)
    eorder = np.argsort(key, kind="stable")
    key_s = key[eorder]
    rel_s = e_rel[eorder].astype(np.int32)
    j_s = e_j[eorder].astype(np.int32)
    grp_start = np.searchsorted(key_s, np.arange(NCORES * NWIN * 2))
    grp_end = np.searchsorted(key_s, np.arange(NCORES * NWIN * 2), side="right")

    idx_arrs = []   # per core: int16 [P, NTILES*P//16] in call order
    s_arrs = []     # per core: bf16 [P, NTILES*W_DST]
    CALL_T = CPW * T_HALF           # tiles per gather call (35)
    CALL_I = CALL_T * P             # idxs per call (4480)
    for c in range(NCORES):
        idx_flat = np.zeros(NTILES * P, dtype=np.int32)  # in gather-call order
        S = np.zeros((P, NTILES * W_DST), dtype=ml_dtypes.bfloat16)
        for ch in range(NCHUNK):
            for h in (0, 1):
                for wl in range(CPW):
                    wi = ch * CPW + wl
                    g = (c * NWIN + wi) * 2 + h
                    lo, hi = grp_start[g], grp_end[g]
                    cnt = hi - lo
                    assert cnt <= T_HALF * P
                    # position within the call's idx stream
                    call_base = (ch * 2 + h) * CALL_I
                    off = call_base + wl * T_HALF * P
                    idx_flat[off:off + cnt] = rel_s[lo:hi]
                    # S: tile slot within msgs buffer = h*CALL_T + wl*T_HALF + kk
                    ks = np.arange(cnt)
                    tile_slot = (h * CALL_T + wl * T_HALF + ks // P)
                    scol = (ch * 2 * CALL_T) * W_DST + tile_slot * W_DST + j_s[lo:hi]
                    S[ks % P, scol] = 1.0
        # wrap idx per call: idx i of call -> [i%16, i//16], replicated on all
        # 8 Q7 core groups (16 partitions each)
        idx16 = np.zeros((P, NTILES * P // 16), dtype=np.int16)
        for call in range(NCHUNK * 2):
            L = idx_flat[call * CALL_I:(call + 1) * CALL_I]
            assert L.max(initial=0) < 32768
            w16 = L.reshape(CALL_I // 16, 16).T.astype(np.int16)
            for k in range(8):
                idx16[16 * k:16 * (k + 1),
                      call * (CALL_I // 16):(call + 1) * (CALL_I // 16)] = w16
        idx_arrs.append(idx16)
        s_arrs.append(S)

    # ---- per-core auxiliary arrays ----
    dinv_slot = np.zeros(NCORES * NPCS, dtype=np.float32)
    valid = slot_node >= 0
    dinv_slot[valid] = dinv[slot_node[valid]]

    dinv_part = []   # [P, NGRP] per core
    dinv_bc = []     # [P, NPCS] per core (replicated over partitions)
    bpool = []       # [P, NGRP*NG] per core
    for c in range(NCORES):
        ds = dinv_slot[c * NPCS:(c + 1) * NPCS]
        dinv_part.append(ds.reshape(NGRP, P).T.copy())
        dinv_bc.append(np.broadcast_to(ds, (P, NPCS)).copy())
        bp = np.zeros((P, NGRP * NG), dtype=np.float32)
        sn = slot_node[c * NPCS:(c + 1) * NPCS]
        for g in range(NGRP):
            for p in range(P):
                v = sn[g * P + p]
                if v >= 0:
                    bp[p, g * NG + batch[v]] = 1.0
        bpool.append(bp)

    cnt_g = np.bincount(batch, minlength=NG).astype(np.float32)
    return dict(perm=perm, slot_node=slot_node, dinv=dinv, cnt_g=cnt_g,
                idx_arrs=idx_arrs, s_arrs=s_arrs, dinv_part=dinv_part,
                dinv_bc=dinv_bc, bpool=bpool)


# ======================= bass program =======================

def _build_program():
    import concourse.bass as bass
    import concourse.tile as tile
    from concourse import bacc, mybir
    from contextlib import ExitStack

    f32 = mybir.dt.float32
    bf16 = mybir.dt.bfloat16
    i16 = mybir.dt.int16

    nc = bacc.Bacc("TRN2", target_bir_lowering=False, debug=False,
                   num_devices=NCORES, enable_asserts=False)

    # ---- IO ----
    xT = nc.dram_tensor("xT", [D_IN, NPCS], f32, kind="ExternalInput").ap()
    W_enc = nc.dram_tensor("W_enc", [D_IN, HID], f32, kind="ExternalInput").ap()
    b_enc = nc.dram_tensor("b_enc", [HID, 1], f32, kind="ExternalInput").ap()
    gcn_W = nc.dram_tensor("gcn_W", [HID, NL * HID], f32, kind="ExternalInput").ap()
    gcn_b = nc.dram_tensor("gcn_b", [HID, NL], f32, kind="ExternalInput").ap()
    W_reg = nc.dram_tensor("W_reg", [HID, 1], f32, kind="ExternalInput").ap()
    idx_in = nc.dram_tensor("idx", [P, NTILES * P // 16], i16, kind="ExternalInput").ap()
    S_in = nc.dram_tensor("S", [P, NTILES * W_DST], bf16, kind="ExternalInput").ap()
    dinvp_in = nc.dram_tensor("dinvp", [P, NGRP], f32, kind="ExternalInput").ap()
    dinvb_in = nc.dram_tensor("dinvb", [P, NPCS], f32, kind="ExternalInput").ap()
    bpool_in = nc.dram_tensor("bpool", [P, NGRP * NG], f32, kind="ExternalInput").ap()
    out_ext = nc.dram_tensor("pool_out", [NG, 1], f32, kind="ExternalOutput").ap()

    # ---- internal DRAM ----
    chunk_d = [nc.dram_tensor(f"chunk{i}", [NPCS, DELEM], bf16).ap()
               for i in range(NL)]
    table_d = [nc.dram_tensor(f"table{i}", [NROWS, DELEM], bf16,
                              addr_space="Shared").ap() for i in range(NL)]

    CALL_T = CPW * T_HALF
    CALL_I = CALL_T * P

    from concourse import library_config
    with tile.TileContext(nc) as tc, ExitStack() as ctx:
        pers = ctx.enter_context(tc.tile_pool(name="pers", bufs=1))
        msgs_p = ctx.enter_context(tc.tile_pool(name="msgs", bufs=2))
        s_p = ctx.enter_context(tc.tile_pool(name="sstream", bufs=2))
        dv_p = ctx.enter_context(tc.tile_pool(name="dvstream", bufs=2))
        stg_p = ctx.enter_context(tc.tile_pool(name="stg", bufs=3))
        ev_p = ctx.enter_context(tc.tile_pool(name="ev", bufs=3))
        ps_win = ctx.enter_context(tc.tile_pool(name="pswin", bufs=4, space="PSUM"))
        ps_tb = ctx.enter_context(tc.tile_pool(name="pstb", bufs=2, space="PSUM"))
        ps_misc = ctx.enter_context(tc.tile_pool(name="psmisc", bufs=1, space="PSUM"))

        # ---- resident tiles ----
        h_bufs = [pers.tile([P, NPCS], f32, tag=f"h{i}", name=f"h{i}") for i in range(2)]
        idx_sb = pers.tile([P, NTILES * P // 16], i16, tag="idx")
        bpool_sb = pers.tile([P, NGRP * NG], f32, tag="bpool")
        dinvp_sb = pers.tile([P, NGRP], f32, tag="dinvp")
        wenc_sb = pers.tile([P, HID], f32, tag="wenc")
        benc_sb = pers.tile([P, 1], f32, tag="benc")
        gcnw_sb = pers.tile([P, NL * HID], f32, tag="gcnw")
        gcnb_sb = pers.tile([P, NL], f32, tag="gcnb")
        wreg_sb = pers.tile([P, 1], f32, tag="wreg")
        zbuf = pers.tile([P, NGRP], f32, tag="zbuf")

        nc.gpsimd.load_library(library_config.mlp)
        nc.sync.dma_start(idx_sb[:], idx_in[:])
        nc.sync.dma_start(bpool_sb[:], bpool_in[:])
        nc.sync.dma_start(dinvp_sb[:], dinvp_in[:])
        nc.sync.dma_start(wenc_sb[:D_IN, :], W_enc[:])
        nc.sync.dma_start(benc_sb[:], b_enc[:])
        nc.sync.dma_start(gcnw_sb[:], gcn_W[:])
        nc.sync.dma_start(gcnb_sb[:], gcn_b[:])
        nc.sync.dma_start(wreg_sb[:], W_reg[:])

        # ---- encoder: h0 = x @ W_enc + b_enc (as [HID, slots]) ----
        h = h_bufs[0]
        ENC_N = 512
        for s0 in range(0, NPCS, ENC_N):
            n = min(ENC_N, NPCS - s0)
            xt = stg_p.tile([P, ENC_N], f32, tag="xt")
            nc.sync.dma_start(xt[:D_IN, :n], xT[:, s0:s0 + n])
            psum = ps_tb.tile([P, ENC_N], f32, space="PSUM", tag="tb", name="encps")
            nc.tensor.matmul(psum[:, :n], lhsT=wenc_sb[:D_IN, :], rhs=xt[:D_IN, :n],
                             start=True, stop=True)
            nc.vector.tensor_scalar_add(h[:, s0:s0 + n], psum[:, :n], benc_sb[:, 0:1])

        # ---- GCN layers ----
        for li in range(NL):
            h_nxt = h_bufs[(li + 1) % 2]
            tbl = table_d[li]
            chk = chunk_d[li]
            Wl = gcnw_sb[:, li * HID:(li + 1) * HID]
            bl = gcnb_sb[:, li:li + 1]

            # table chunk build: rows = dinv * (h.T @ W) as bf16 hi|lo
            for g in range(NGRP):
                pt = ps_tb.tile([P, HID], f32, space="PSUM", tag="tb")
                nc.tensor.matmul(pt[:], lhsT=h[:, g * P:(g + 1) * P], rhs=Wl,
                                 start=True, stop=True)
                tmp = stg_p.tile([P, HID], f32, tag="tmp")
                nc.vector.tensor_scalar_mul(tmp[:], pt[:], dinvp_sb[:, g:g + 1])
                stg = stg_p.tile([P, DELEM], bf16, tag="stg")
                nc.vector.tensor_copy(stg[:, 0:HID], tmp[:])
                nc.vector.tensor_tensor(out=stg[:, HID:DELEM], in0=tmp[:],
                                        in1=stg[:, 0:HID],
                                        op=mybir.AluOpType.subtract)
                nc.sync.dma_start(chk[g * P:(g + 1) * P, :], stg[:])

            nc.gpsimd.collective_compute(
                "AllGather", mybir.AluOpType.bypass,
                replica_groups=[list(range(NCORES))],
                ins=[chk[:]], outs=[tbl[:]],
            )

            # gather + scatter chunks
            for ch in range(NCHUNK):
                mgs = msgs_p.tile([P, 2 * CALL_T * DELEM], bf16, tag="m")
                for hh in (0, 1):
                    call = ch * 2 + hh
                    src_ap = tbl[0:HALF_ROW, :] if hh == 0 else tbl[HALF_ROW:NROWS, :]
                    nc.gpsimd.dma_gather(
                        out_ap=mgs[:, hh * CALL_T * DELEM:(hh + 1) * CALL_T * DELEM]
                            .rearrange("p (k d) -> p k d", d=DELEM),
                        in_ap=src_ap,
                        idxs_ap=idx_sb[:, call * (CALL_I // 16):(call + 1) * (CALL_I // 16)],
                        num_idxs=CALL_I,
                        num_idxs_reg=CALL_I,
                        elem_size=DELEM,
                        single_packet=False,
                    )
                st = s_p.tile([P, 2 * CALL_T * W_DST], bf16, tag="s")
                nc.sync.dma_start(st[:], S_in[:, ch * 2 * CALL_T * W_DST:(ch + 1) * 2 * CALL_T * W_DST])
                dvt = dv_p.tile([P, CPW * W_DST], f32, tag="dv")
                nc.sync.dma_start(dvt[:], dinvb_in[:, ch * CPW * W_DST:(ch + 1) * CPW * W_DST])

                for wl in range(CPW):
                    wi = ch * CPW + wl
                    pw = ps_win.tile([P, W_DST], f32, space="PSUM", tag="win")
                    for hh in (0, 1):
                        for kk in range(T_HALF):
                            slot = hh * CALL_T + wl * T_HALF + kk
                            srow = slot * W_DST
                            first = (hh == 0 and kk == 0)
                            last = (hh == 1 and kk == T_HALF - 1)
                            nc.tensor.matmul(
                                pw[:], lhsT=mgs[:, slot * DELEM:slot * DELEM + HID],
                                rhs=st[:, srow:srow + W_DST],
                                start=first, stop=False)
                            nc.tensor.matmul(
                                pw[:], lhsT=mgs[:, slot * DELEM + HID:(slot + 1) * DELEM],
                                rhs=st[:, srow:srow + W_DST],
                                start=False, stop=last)
                    sc = ev_p.tile([P, W_DST], f32, tag="sc")
                    nc.vector.tensor_tensor(out=sc[:], in0=pw[:],
                                            in1=dvt[:, wl * W_DST:(wl + 1) * W_DST],
                                            op=mybir.AluOpType.mult)
                    nc.vector.tensor_scalar(
                        out=h_nxt[:, wi * W_DST:(wi + 1) * W_DST], in0=sc[:],
                        scalar1=bl, scalar2=0.0,
                        op0=mybir.AluOpType.add, op1=mybir.AluOpType.max)
            h = h_nxt

        # ---- regression + pool ----
        for g in range(NGRP):
            pz = ps_misc.tile([P, 1], f32, space="PSUM", tag="z", bufs=1)
            nc.tensor.matmul(pz[:], lhsT=h[:, g * P:(g + 1) * P], rhs=wreg_sb[:],
                             start=True, stop=True)
            nc.vector.tensor_copy(zbuf[:, g:g + 1], pz[:])
        pp = ps_misc.tile([NG, 1], f32, space="PSUM", tag="pool", bufs=1)
        for g in range(NGRP):
            nc.tensor.matmul(pp[:], lhsT=bpool_sb[:, g * NG:(g + 1) * NG],
                             rhs=zbuf[:, g:g + 1],
                             start=(g == 0), stop=(g == NGRP - 1))
        outt = ev_p.tile([NG, 1], f32, tag="out")
        nc.vector.tensor_copy(outt[:], pp[:])
        nc.sync.dma_start(out_ext[:], outt[:])

    nc.compile()
    return nc


# ======================= entry point =======================

def kernel(x, edge_index, batch, W_enc, b_enc, gcn_W, gcn_b, W_reg, b_reg):
    x = np.asarray(x, dtype=np.float32)
    edge_index = np.asarray(edge_index)
    batch = np.asarray(batch)
    W_enc = np.asarray(W_enc, dtype=np.float32)
    b_enc = np.asarray(b_enc, dtype=np.float32)
    gcn_W = np.asarray(gcn_W, dtype=np.float32)
    gcn_b = np.asarray(gcn_b, dtype=np.float32)
    W_reg = np.asarray(W_reg, dtype=np.float32)
    b_reg = np.asarray(b_reg, dtype=np.float32)

    key = (edge_index.tobytes(), batch.tobytes())
    pk = hash(key)
    if pk not in _cache:
        pre = _preprocess(edge_index, batch)
        nc = _build_program()
        _cache.clear()
        _cache[pk] = (pre, nc)
    pre, nc = _cache[pk]

    in_maps = _make_inputs(pre, x, W_enc, b_enc, gcn_W, gcn_b, W_reg)

    from concourse.bass_utils import run_bass_kernel_spmd
    res = run_bass_kernel_spmd(nc, in_maps, core_ids=list(range(NCORES)),
                               trace=bool(int(os.environ.get("GCN_TRACE", "0"))))
    if res.exec_time_ns is not None:
        print(f"HW exec time: {res.exec_time_ns} ns", flush=True)

    pool = np.zeros((NG, 1), dtype=np.float32)
    for c in range(NCORES):
        pool += res.results[c]["pool_out"]
    out = pool / np.maximum(pre["cnt_g"], 1.0)[:, None] + b_reg
    return out.astype(np.float32)


def _make_inputs(pre, x, W_enc, b_enc, gcn_W, gcn_b, W_reg):
    in_maps = []
    slot_node = pre["slot_node"]
    for c in range(NCORES):
        sn = slot_node[c * NPCS:(c + 1) * NPCS]
        xTc = np.zeros((D_IN, NPCS), dtype=np.float32)
        valid = sn >= 0
        xTc[:, valid] = x[sn[valid]].T
        in_maps.append({
            "xT": xTc,
            "W_enc": W_enc,
            "b_enc": b_enc.reshape(HID, 1),
            "gcn_W": np.concatenate([gcn_W[l] for l in range(NL)], axis=1),
            "gcn_b": gcn_b.T.copy().reshape(HID, NL),
            "W_reg": W_reg.reshape(HID, 1),
            "idx": pre["idx_arrs"][c],
            "S": pre["s_arrs"][c],
            "dinvp": pre["dinv_part"][c],
            "dinvb": pre["dinv_bc"][c],
            "bpool": pre["bpool"][c],
        })
    return in_maps


# expose pieces for test harness
def build_all(inputs):
    pre = _preprocess(np.asarray(inputs["edge_index"]), np.asarray(inputs["batch"]))
    nc = _build_program()
    in_maps = _make_inputs(pre, np.asarray(inputs["x"], dtype=np.float32),
                           np.asarray(inputs["W_enc"], dtype=np.float32),
                           np.asarray(inputs["b_enc"], dtype=np.float32),
                           np.asarray(inputs["gcn_W"], dtype=np.float32),
                           np.asarray(inputs["gcn_b"], dtype=np.float32),
                           np.asarray(inputs["W_reg"], dtype=np.float32))
    return pre, nc, in_maps



# revision 2
# speedup vs baseline: 1.0400x; 1.0400x over previous
"""Trainium2 Bass kernel v2 for nn_GCNNet (3-layer GCN, 50k nodes, 800k edges,
HID=128, 64 graphs) sharded across 8 NeuronCores.

Changes vs v1 baseline:
- bf16-only node-feature table (256B rows instead of hi|lo 512B): halves
  gather bytes, scatter matmuls, and AllGather traffic. rel-err target is
  2e-2; bf16 quantization lands ~1e-3.
- dma_gather issued as prepare_only + trigger_dma on 2 SWDGE queues so the
  gpsimd engine no longer blocks for the DMA duration; transfers from
  consecutive calls overlap.
- S (segment-sum selection) matrices are expanded on-device from a per-edge
  dst-column byte via a broadcast is_equal, instead of streaming 16MB/layer
  of precomputed S from HBM.
- dinv_dst vector and pool matrix resident in SBUF; encoder inputs in bf16.
- dst-side norm folded: h is stored "raw" (pre dinv_dst scale); the scale is
  applied via dinv^2 at the next table build and via dinv at the final
  regression. Eviction = (psum * dinv_bcast) + bias, relu ... kept explicit
  (2 DVE ops) -- actually h stored raw means eviction skips the dinv mult.
"""
import os
import numpy as np
import ml_dtypes

# ---- problem constants (hardcoded; kernel.py must be self-contained) ----
N = 50000
E = 800000
D_IN = 100
HID = 128
NL = 3
NG = 64

NCORES = 8
P = 128
W_DST = 64             # dst columns per window
NWIN = 98              # windows per core
NPCS = NWIN * W_DST    # 6272 slots per core
NGRP = NPCS // P       # 49 groups of 128 slots
T_HALF = 5             # K-tiles per src-half per window
T_WIN = 2 * T_HALF     # 10 tiles per window
NTILES = NWIN * T_WIN  # 980 tiles per core
NROWS = NCORES * NPCS  # 50176 table rows
HALF_ROW = NROWS // 2  # 25088 = cores 0..3
CPW = 7                # windows per gather/scatter chunk
NCHUNK = NWIN // CPW   # 14 chunks
DELEM = HID            # 128 bf16 per table row = 256B

CALL_T = CPW * T_HALF           # tiles per gather call (35)
CALL_I = CALL_T * P             # idxs per call (4480)

_cache = {}


# ======================= host preprocessing =======================

def _snake(order, nbins):
    """Assign sorted items to bins in snake order; returns bin id per item."""
    n = len(order)
    assert n % nbins == 0
    rounds = n // nbins
    cols = np.tile(np.arange(nbins), (rounds, 1))
    cols[1::2] = cols[1::2][:, ::-1]
    bin_of = np.empty(n, dtype=np.int64)
    bin_of[order] = cols.ravel()
    return bin_of


def _preprocess(edge_index, batch):
    src = np.asarray(edge_index[0], dtype=np.int64)
    dst = np.asarray(edge_index[1], dtype=np.int64)
    batch = np.asarray(batch, dtype=np.int64)

    deg = (np.bincount(dst, minlength=N) + 1).astype(np.float32)
    dinv = (1.0 / np.sqrt(deg)).astype(np.float32)

    # ---- node -> core assignment, snake-balanced by in-degree(+1) ----
    w = deg.astype(np.int64)
    order = np.argsort(-w, kind="stable")
    node_core = _snake(order, NCORES).astype(np.int32)

    # half A = src owned by cores 0..3; self-loops are computed locally
    # (not gathered), so they do not contribute to edge counts.
    src_half = (node_core[src] >= 4).astype(np.int64)
    a_cnt = np.bincount(dst[src_half == 0], minlength=N)
    b_cnt = np.bincount(dst[src_half == 1], minlength=N)

    # ---- per-core window assignment (snake by total, repair half caps) ----
    CAP = T_HALF * P  # 640 per half
    node_slot = np.full(N, -1, dtype=np.int64)
    slot_node = np.full(NCORES * NPCS, -1, dtype=np.int64)
    wa_ranks = []
    wb_ranks = []
    tot = a_cnt + b_cnt
    LCAP = 4 * P  # light windows: 4 tiles per half
    for c in range(NCORES):
        nodes = np.nonzero(node_core == c)[0]
        npad = NPCS - len(nodes)
        # pad with fake node ids (-1) of zero weight
        ww = np.concatenate([tot[nodes], np.zeros(npad, dtype=np.int64)])
        ids = np.concatenate([nodes, np.full(npad, -1, dtype=np.int64)])
        order_c = np.argsort(-ww, kind="stable")
        # skewed first-fit: N_HEAVY windows cap 5 tiles/half, rest 4
        for N_HEAVY in (16, 24, 32, 48, NWIN):
            capA = np.full(NWIN, LCAP, dtype=np.int64)
            capB = np.full(NWIN, LCAP, dtype=np.int64)
            capA[:N_HEAVY] = CAP
            capB[:N_HEAVY] = CAP
            wa = np.zeros(NWIN, dtype=np.int64)
            wb = np.zeros(NWIN, dtype=np.int64)
            wc = np.zeros(NWIN, dtype=np.int64)
            win_of = np.empty(NPCS, dtype=np.int64)
            ok = True
            RES = 64
            for i in order_c:
                v = ids[i]
                av = a_cnt[v] if v >= 0 else 0
                bv = b_cnt[v] if v >= 0 else 0
                feas = np.nonzero((wc < W_DST) & (wa + av <= capA - RES)
                                  & (wb + bv <= capB - RES))[0]
                if len(feas) == 0:
                    feas = np.nonzero((wc < W_DST) & (wa + av <= capA)
                                      & (wb + bv <= capB))[0]
                if len(feas) == 0:
                    ok = False
                    break
                j = feas[np.argmax((wa[feas] + wb[feas]) * 100 - wc[feas])]
                win_of[i] = j
                wa[j] += av; wb[j] += bv; wc[j] += 1
            if ok:
                break
        else:
            raise RuntimeError("skew packing overflow")
        # relabel windows by decreasing load so window id == rank on every
        # core (SPMD-static variable-tile template)
        rank_of = np.empty(NWIN, dtype=np.int64)
        rank_of[np.argsort(-(wa + wb), kind="stable")] = np.arange(NWIN)
        win_of = rank_of[win_of]
        wa_r = np.zeros(NWIN, dtype=np.int64)
        wb_r = np.zeros(NWIN, dtype=np.int64)
        for i in range(NPCS):
            v = ids[i]
            if v >= 0:
                wa_r[win_of[i]] += a_cnt[v]
                wb_r[win_of[i]] += b_cnt[v]
        wa_ranks.append(wa_r)
        wb_ranks.append(wb_r)
        # slot within window by arrival order
        slot_in_win = np.zeros(NPCS, dtype=np.int64)
        cnt_w = np.zeros(NWIN, dtype=np.int64)
        for i in range(NPCS):
            slot_in_win[i] = cnt_w[win_of[i]]
            cnt_w[win_of[i]] += 1
        assert cnt_w.max() <= W_DST
        glob = c * NPCS + win_of * W_DST + slot_in_win
        real = ids >= 0
        node_slot[ids[real]] = glob[real]
        slot_node[glob[real]] = ids[real]

    perm = node_slot
    # per-window tile template (shared across cores)
    wa_ranks = np.stack(wa_ranks)
    wb_ranks = np.stack(wb_ranks)
    tA = np.maximum(np.ceil(wa_ranks.max(axis=0) / P).astype(np.int64), 1)
    tB = np.maximum(np.ceil(wb_ranks.max(axis=0) / P).astype(np.int64), 1)

    # ---- static chunk/call template from tile counts ----
    wic = [[wi for wi in range(NWIN) if wi % NCHUNK == ch] for ch in range(NCHUNK)]
    offA = np.zeros(NWIN, dtype=np.int64)  # tile offset of window within its A call
    offB = np.zeros(NWIN, dtype=np.int64)
    callA_T = np.zeros(NCHUNK, dtype=np.int64)
    callB_T = np.zeros(NCHUNK, dtype=np.int64)
    for ch in range(NCHUNK):
        a = b = 0
        for wi in wic[ch]:
            offA[wi] = a; a += tA[wi]
            offB[wi] = b; b += tB[wi]
        callA_T[ch] = a
        callB_T[ch] = b
    # idx stream element offset of each call (order: ch asc, half A then B)
    call_off = np.zeros((NCHUNK, 2), dtype=np.int64)
    tile_off = np.zeros((NCHUNK, 2), dtype=np.int64)  # tile-stream offsets
    acc_e = acc_t = 0
    for ch in range(NCHUNK):
        for h in (0, 1):
            call_off[ch, h] = acc_e
            tile_off[ch, h] = acc_t
            t = int(callA_T[ch] if h == 0 else callB_T[ch])
            acc_e += t * P
            acc_t += t
    NTILES_TOT = int(acc_t)
    tmpl = dict(tA=tA, tB=tB, wic=wic, offA=offA, offB=offB,
                callA_T=callA_T, callB_T=callB_T, call_off=call_off,
                tile_off=tile_off, ntiles=NTILES_TOT,
                mot=int(max(callA_T.max(), callB_T.max())),
                smax=int((callA_T + callB_T).max()))

    # ---- per-core edge slot assignment (no self-loops) ----
    all_src = src
    all_dst = dst
    e_row = perm[all_src]
    e_half = (e_row >= HALF_ROW).astype(np.int64)
    e_rel = e_row - e_half * HALF_ROW
    e_dslot = perm[all_dst]
    e_core = e_dslot // NPCS
    e_local = e_dslot % NPCS
    e_win = e_local // W_DST
    e_j = e_local % W_DST

    key = ((e_core * NWIN + e_win) * 2 + e_half)
    eorder = np.argsort(key, kind="stable")
    key_s = key[eorder]
    rel_s = e_rel[eorder].astype(np.int32)
    j_s = e_j[eorder].astype(np.int32)
    grp_start = np.searchsorted(key_s, np.arange(NCORES * NWIN * 2))
    grp_end = np.searchsorted(key_s, np.arange(NCORES * NWIN * 2), side="right")

    idx_arrs = []   # per core: int16 [P, NTILES_TOT*P//16] in call order
    dstc_arrs = []  # per core: bf16 [P, NTILES_TOT] dst column per slot (64=pad)
    for c in range(NCORES):
        idx_flat = np.zeros(NTILES_TOT * P, dtype=np.int32)
        dstc_flat = np.full(NTILES_TOT * P, W_DST, dtype=np.float32)
        for ch in range(NCHUNK):
            for h in (0, 1):
                offw = offA if h == 0 else offB
                tw = tA if h == 0 else tB
                for wi in wic[ch]:
                    g = (c * NWIN + wi) * 2 + h
                    lo, hi = grp_start[g], grp_end[g]
                    cnt = hi - lo
                    assert cnt <= tw[wi] * P, (c, ch, h, wi, cnt, tw[wi])
                    off = call_off[ch, h] + offw[wi] * P
                    idx_flat[off:off + cnt] = rel_s[lo:hi]
                    dstc_flat[off:off + cnt] = j_s[lo:hi]
        # idx wrap per call: idx i of call -> [i%16, i//16], replicated x8
        idx16 = np.zeros((P, NTILES_TOT * P // 16), dtype=np.int16)
        for ch in range(NCHUNK):
            for h in (0, 1):
                L = int((callA_T[ch] if h == 0 else callB_T[ch]) * P)
                e0 = int(call_off[ch, h])
                blk = idx_flat[e0:e0 + L].reshape(L // 16, 16).T.astype(np.int16)
                for k in range(8):
                    idx16[16 * k:16 * (k + 1), e0 // 16:(e0 + L) // 16] = blk
        idx_arrs.append(idx16)
        dstc_arrs.append(dstc_flat.reshape(NTILES_TOT, P).T
                         .astype(ml_dtypes.bfloat16).copy())

    # ---- per-core auxiliary arrays ----
    dinv_slot = np.zeros(NCORES * NPCS, dtype=np.float32)
    valid = slot_node >= 0
    dinv_slot[valid] = dinv[slot_node[valid]]

    dinvp = []      # [P, NGRP] f32 (power 1, layer-0 table build + final z)
    dinvp2 = []     # [P, NGRP] f32 (power 2, later table builds)
    dinv_bc = []    # [P, NPCS] bf16 replicated (dst-scale at eviction)
    bpool = []      # [P, NGRP*NG] f32
    for c in range(NCORES):
        ds = dinv_slot[c * NPCS:(c + 1) * NPCS]
        dp = ds.reshape(NGRP, P).T.copy()
        dinvp.append(dp)
        dinvp2.append((dp * dp).copy())
        dinv_bc.append(np.broadcast_to(ds.astype(ml_dtypes.bfloat16), (P, NPCS)).copy())
        sn = slot_node[c * NPCS:(c + 1) * NPCS]
        bp = np.zeros((P, NGRP * NG), dtype=np.float32)
        g_idx = np.arange(NPCS) // P
        p_idx = np.arange(NPCS) % P
        ok = sn >= 0
        bp[p_idx[ok], g_idx[ok] * NG + batch[sn[ok]]] = 1.0
        bpool.append(bp)

    cnt_g = np.bincount(batch, minlength=NG).astype(np.float32)
    return dict(perm=perm, slot_node=slot_node, dinv=dinv, cnt_g=cnt_g,
                idx_arrs=idx_arrs, dstc_arrs=dstc_arrs, dinvp=dinvp,
                dinvp2=dinvp2, dinv_bc=dinv_bc, bpool=bpool, tmpl=tmpl)


# ======================= bass program =======================

def _build_program(tmpl):
    import concourse.bass as bass
    import concourse.tile as tile
    from concourse import bacc, mybir
    from contextlib import ExitStack

    tA, tB, wic = tmpl["tA"], tmpl["tB"], tmpl["wic"]
    offA, offB = tmpl["offA"], tmpl["offB"]
    callA_T, callB_T = tmpl["callA_T"], tmpl["callB_T"]
    call_off, tile_off = tmpl["call_off"], tmpl["tile_off"]
    NTILES_TOT, MOT, SMAX = tmpl["ntiles"], tmpl["mot"], tmpl["smax"]

    f32 = mybir.dt.float32
    bf16 = mybir.dt.bfloat16
    i16 = mybir.dt.int16

    NQ = int(os.environ.get("GCN_NQ", "2"))
    nc = bacc.Bacc("TRN2", target_bir_lowering=False, debug=False,
                   num_devices=NCORES, enable_asserts=False,
                   num_swdge_queues=NQ)

    # ---- IO ----
    xT = nc.dram_tensor("xT", [D_IN, NPCS], bf16, kind="ExternalInput").ap()
    W_enc = nc.dram_tensor("W_enc", [D_IN, HID], bf16, kind="ExternalInput").ap()
    b_enc = nc.dram_tensor("b_enc", [HID, 1], f32, kind="ExternalInput").ap()
    gcn_W = nc.dram_tensor("gcn_W", [HID, NL * HID], bf16, kind="ExternalInput").ap()
    gcn_b = nc.dram_tensor("gcn_b", [HID, NL], f32, kind="ExternalInput").ap()
    W_reg = nc.dram_tensor("W_reg", [HID, 1], bf16, kind="ExternalInput").ap()
    idx_in = nc.dram_tensor("idx", [P, NTILES_TOT * P // 16], i16, kind="ExternalInput").ap()
    dstc_in = nc.dram_tensor("dstc", [P, NTILES_TOT], bf16, kind="ExternalInput").ap()
    iota_in = nc.dram_tensor("iota", [P, W_DST], bf16, kind="ExternalInput").ap()
    dinvp_in = nc.dram_tensor("dinvp", [P, NGRP], f32, kind="ExternalInput").ap()
    dinvp2_in = nc.dram_tensor("dinvp2", [P, NGRP], f32, kind="ExternalInput").ap()
    dinvb_in = nc.dram_tensor("dinvb", [P, NPCS], bf16, kind="ExternalInput").ap()
    bpool_in = nc.dram_tensor("bpool", [P, NGRP * NG], f32, kind="ExternalInput").ap()
    out_ext = nc.dram_tensor("pool_out", [NG, 1], f32, kind="ExternalOutput").ap()

    # ---- internal DRAM ----
    chunk_d = [nc.dram_tensor(f"chunk{i}", [NPCS, DELEM], bf16).ap()
               for i in range(NL)]
    table_d = [nc.dram_tensor(f"table{i}", [NROWS, DELEM], bf16,
                              addr_space="Shared").ap() for i in range(NL)]

    from concourse import library_config
    with tile.TileContext(nc) as tc, ExitStack() as ctx:
        pers = ctx.enter_context(tc.tile_pool(name="pers", bufs=1))
        msgs_p = ctx.enter_context(tc.tile_pool(name="msgs", bufs=4))
        s_p = ctx.enter_context(tc.tile_pool(name="sstream", bufs=2))
        stg_p = ctx.enter_context(tc.tile_pool(name="stg", bufs=3))
        ev_p = ctx.enter_context(tc.tile_pool(name="ev", bufs=3))
        ps_win = ctx.enter_context(tc.tile_pool(name="pswin", bufs=4, space="PSUM"))
        ps_tb = ctx.enter_context(tc.tile_pool(name="pstb", bufs=2, space="PSUM"))
        ps_misc = ctx.enter_context(tc.tile_pool(name="psmisc", bufs=1, space="PSUM"))

        # ---- resident tiles ----
        h_bufs = [pers.tile([P, NPCS], bf16, tag=f"h{i}", name=f"h{i}") for i in range(2)]
        hs_sb = pers.tile([P, NPCS], bf16, tag="hs")
        idx_sb = pers.tile([P, NTILES_TOT * P // 16], i16, tag="idx")
        dstc_sb = pers.tile([P, NTILES_TOT], bf16, tag="dstc")
        iota_sb = pers.tile([P, W_DST], bf16, tag="iota")
        bpool_sb = pers.tile([P, NGRP * NG], f32, tag="bpool")
        dinvp_sb = pers.tile([P, NGRP], f32, tag="dinvp")
        dinvp2_sb = pers.tile([P, NGRP], f32, tag="dinvp2")
        dinvb_sb = pers.tile([P, NPCS], bf16, tag="dinvb")
        wenc_sb = pers.tile([P, HID], bf16, tag="wenc")
        benc_sb = pers.tile([P, 1], f32, tag="benc")
        gcnw_sb = pers.tile([P, NL * HID], bf16, tag="gcnw")
        gcnb_sb = pers.tile([P, NL], f32, tag="gcnb")
        wreg_sb = pers.tile([P, 1], bf16, tag="wreg")
        zbuf = pers.tile([P, NGRP], f32, tag="zbuf")

        nc.gpsimd.load_library(library_config.mlp)
        nc.sync.dma_start(idx_sb[:], idx_in[:])
        nc.sync.dma_start(dstc_sb[:], dstc_in[:])
        nc.sync.dma_start(iota_sb[:], iota_in[:])
        nc.sync.dma_start(bpool_sb[:], bpool_in[:])
        nc.sync.dma_start(dinvp_sb[:], dinvp_in[:])
        nc.sync.dma_start(dinvp2_sb[:], dinvp2_in[:])
        nc.sync.dma_start(dinvb_sb[:], dinvb_in[:])
        nc.sync.dma_start(wenc_sb[:D_IN, :], W_enc[:])
        nc.sync.dma_start(benc_sb[:], b_enc[:])
        nc.sync.dma_start(gcnw_sb[:], gcn_W[:])
        nc.sync.dma_start(gcnb_sb[:], gcn_b[:])
        nc.sync.dma_start(wreg_sb[:], W_reg[:])

        # gather call plumbing: persistent round-robin msgs buffers (the tile
        # pool scheduler does not insert WAR edges for deferred prep-mode
        # gather writes), rotating sems (a fast ring on call N+1 must not
        # mask a slow ring on call N via a shared counter), explicit
        # consumer-side wait_ge on the PE queue.
        NBUF = NQ + 1  # in-flight gather buffers per half
        SEMS_PER_Q = 8
        sem_q = [[nc.alloc_semaphore(f"gq{q}_{i}") for i in range(SEMS_PER_Q)]
                 for q in range(NQ)]
        sem_ctr = [0] * NQ
        mbufs = [[pers.tile([P, MOT * DELEM], bf16, tag=f"mb{h}_{i}",
                            name=f"mb{h}_{i}") for i in range(NBUF)]
                 for h in range(2)]

        def gather_call(tbl_half_ap, e0, nidx, q, buf):
            ctr = sem_ctr[q]
            sem_ctr[q] += 1
            sq = sem_q[q][ctr % SEMS_PER_Q]
            tgt = 16 * (ctr // SEMS_PER_Q + 1)
            nt = nidx // P
            nc.gpsimd.dma_gather(
                out_ap=buf[:, 0:nt * DELEM].rearrange("p (k d) -> p k d", d=DELEM),
                in_ap=tbl_half_ap,
                idxs_ap=idx_sb[:, e0 // 16:(e0 + nidx) // 16],
                num_idxs=nidx,
                num_idxs_reg=nidx,
                elem_size=DELEM,
                single_packet=False,
                queue_num=q,
                prepare_only=True,
                sem=sq,
            )
            nc.gpsimd.trigger_dma(count=1, queue_num=q)
            return sq, tgt

        # ---- encoder: h0 = x @ W_enc + b_enc (as [HID, slots], bf16) ----
        h = h_bufs[0]
        ENC_N = 512
        for s0 in range(0, NPCS, ENC_N):
            n = min(ENC_N, NPCS - s0)
            xt = stg_p.tile([P, ENC_N], bf16, tag="xt")
            nc.sync.dma_start(xt[:D_IN, :n], xT[:, s0:s0 + n])
            psum = ps_tb.tile([P, ENC_N], f32, space="PSUM", tag="tb", name="encps")
            nc.tensor.matmul(psum[:, :n], lhsT=wenc_sb[:D_IN, :], rhs=xt[:D_IN, :n],
                             start=True, stop=True)
            nc.vector.tensor_scalar_add(h[:, s0:s0 + n], psum[:, :n], benc_sb[:, 0:1])

        # ---- GCN layers ----
        # h0 holds true h (encoder); later h holds "raw" h (pre dinv_dst
        # scale) and the dst scale is folded into the next build via dinv^2.
        for li in range(NL):
            h_nxt = h_bufs[(li + 1) % 2]
            tbl = table_d[li]
            chk = chunk_d[li]
            Wl = gcnw_sb[:, li * HID:(li + 1) * HID]
            bl = gcnb_sb[:, li:li + 1]
            dscale = dinvp_sb

            # table chunk build: rows = dscale * (h.T @ W) as bf16
            for g in range(NGRP):
                pt = ps_tb.tile([P, HID], f32, space="PSUM", tag="tb")
                nc.tensor.matmul(pt[:], lhsT=h[:, g * P:(g + 1) * P], rhs=Wl,
                                 start=True, stop=True)
                stg = stg_p.tile([P, DELEM], bf16, tag="stg")
                nc.vector.tensor_scalar_mul(stg[:], pt[:], dscale[:, g:g + 1])
                nc.sync.dma_start(chk[g * P:(g + 1) * P, :], stg[:])

            nc.gpsimd.collective_compute(
                "AllGather", mybir.AluOpType.bypass,
                replica_groups=[list(range(NCORES))],
                ins=[chk[:]], outs=[tbl[:]],
            )

            # hs = h * dinv_dst (self-loop contributions, computed locally)
            nc.vector.tensor_tensor(out=hs_sb[:], in0=h[:], in1=dinvb_sb[:],
                                    op=mybir.AluOpType.mult)

            # gather + scatter chunks
            for ch in range(NCHUNK):
                cAT = int(callA_T[ch])
                cBT = int(callB_T[ch])
                mg = [None, None]
                waits = []
                for hh in (0, 1):
                    src_ap = tbl[0:HALF_ROW, :] if hh == 0 else tbl[HALF_ROW:NROWS, :]
                    mg[hh] = mbufs[hh][ch % NBUF]
                    nidx = (cAT if hh == 0 else cBT) * P
                    waits.append(gather_call(src_ap, int(call_off[ch, hh]), nidx,
                                             (ch * 2 + hh) % NQ, mg[hh]))

                # expand S tiles for this chunk from dst columns
                nst = cAT + cBT
                st = s_p.tile([P, SMAX * W_DST], bf16, tag="s")
                nc.vector.tensor_tensor(
                    out=st[:, 0:nst * W_DST].rearrange("p (t j) -> p t j", j=W_DST),
                    in0=iota_sb[:, 0:W_DST].unsqueeze(1)
                        .broadcast_to([P, nst, W_DST]),
                    in1=dstc_sb[:, int(tile_off[ch, 0]):int(tile_off[ch, 0]) + nst]
                        .unsqueeze(2).broadcast_to([P, nst, W_DST]),
                    op=mybir.AluOpType.is_equal)

                for sq, tgt in waits:
                    nc.tensor.wait_ge(sq, tgt)
                for wi in wic[ch]:
                    pw = ps_win.tile([P, W_DST], f32, space="PSUM", tag="win")
                    nc.tensor.matmul(
                        pw[:], lhsT=Wl,
                        rhs=hs_sb[:, wi * W_DST:(wi + 1) * W_DST],
                        start=True, stop=False)
                    for hh in (0, 1):
                        tw = int((tA if hh == 0 else tB)[wi])
                        ow = int((offA if hh == 0 else offB)[wi])
                        sbase = (0 if hh == 0 else cAT) + ow
                        for kk in range(tw):
                            tloc = ow + kk
                            srow = (sbase + kk) * W_DST
                            last = (hh == 1 and kk == tw - 1)
                            nc.tensor.matmul(
                                pw[:], lhsT=mg[hh][:, tloc * DELEM:(tloc + 1) * DELEM],
                                rhs=st[:, srow:srow + W_DST],
                                start=False, stop=last)
                    sc = ev_p.tile([P, W_DST], f32, tag="sc")
                    nc.vector.tensor_tensor(out=sc[:], in0=pw[:],
                                            in1=dinvb_sb[:, wi * W_DST:(wi + 1) * W_DST],
                                            op=mybir.AluOpType.mult)
                    nc.vector.tensor_scalar(
                        out=h_nxt[:, wi * W_DST:(wi + 1) * W_DST], in0=sc[:],
                        scalar1=bl, scalar2=0.0,
                        op0=mybir.AluOpType.add, op1=mybir.AluOpType.max)
            h = h_nxt

        # ---- regression + pool ----
        for g in range(NGRP):
            pz = ps_misc.tile([P, 1], f32, space="PSUM", tag="z", bufs=1)
            nc.tensor.matmul(pz[:], lhsT=h[:, g * P:(g + 1) * P], rhs=wreg_sb[:],
                             start=True, stop=True)
            nc.vector.tensor_copy(zbuf[:, g:g + 1], pz[:])
        pp = ps_misc.tile([NG, 1], f32, space="PSUM", tag="pool", bufs=1)
        for g in range(NGRP):
            nc.tensor.matmul(pp[:], lhsT=bpool_sb[:, g * NG:(g + 1) * NG],
                             rhs=zbuf[:, g:g + 1],
                             start=(g == 0), stop=(g == NGRP - 1))
        outt = ev_p.tile([NG, 1], f32, tag="out")
        nc.vector.tensor_copy(outt[:], pp[:])
        nc.sync.dma_start(out_ext[:], outt[:])

    nc.compile()
    return nc


# ======================= entry point =======================

def kernel(x, edge_index, batch, W_enc, b_enc, gcn_W, gcn_b, W_reg, b_reg):
    x = np.asarray(x, dtype=np.float32)
    edge_index = np.asarray(edge_index)
    batch = np.asarray(batch)
    W_enc = np.asarray(W_enc, dtype=np.float32)
    b_enc = np.asarray(b_enc, dtype=np.float32)
    gcn_W = np.asarray(gcn_W, dtype=np.float32)
    gcn_b = np.asarray(gcn_b, dtype=np.float32)
    W_reg = np.asarray(W_reg, dtype=np.float32)
    b_reg = np.asarray(b_reg, dtype=np.float32)

    key = (edge_index.tobytes(), batch.tobytes())
    pk = hash(key)
    if pk not in _cache:
        pre = _preprocess(edge_index, batch)
        nc = _build_program(pre["tmpl"])
        _cache.clear()
        _cache[pk] = (pre, nc)
    pre, nc = _cache[pk]

    in_maps = _make_inputs(pre, x, W_enc, b_enc, gcn_W, gcn_b, W_reg)

    from concourse.bass_utils import run_bass_kernel_spmd
    res = run_bass_kernel_spmd(nc, in_maps, core_ids=list(range(NCORES)),
                               trace=bool(int(os.environ.get("GCN_TRACE", "0"))))
    if res.exec_time_ns is not None:
        print(f"HW exec time: {res.exec_time_ns} ns", flush=True)

    pool = np.zeros((NG, 1), dtype=np.float32)
    for c in range(NCORES):
        pool += res.results[c]["pool_out"]
    out = pool / np.maximum(pre["cnt_g"], 1.0)[:, None] + b_reg
    return out.astype(np.float32)


def _make_inputs(pre, x, W_enc, b_enc, gcn_W, gcn_b, W_reg):
    bfl = ml_dtypes.bfloat16
    in_maps = []
    slot_node = pre["slot_node"]
    iota = np.broadcast_to(np.arange(W_DST, dtype=np.float32), (P, W_DST))
    iota = iota.astype(bfl)
    for c in range(NCORES):
        sn = slot_node[c * NPCS:(c + 1) * NPCS]
        xTc = np.zeros((D_IN, NPCS), dtype=bfl)
        valid = sn >= 0
        xTc[:, valid] = x[sn[valid]].T.astype(bfl)
        in_maps.append({
            "xT": xTc,
            "W_enc": W_enc.astype(bfl),
            "b_enc": b_enc.reshape(HID, 1),
            "gcn_W": np.concatenate([gcn_W[l] for l in range(NL)], axis=1).astype(bfl),
            "gcn_b": gcn_b.T.copy().reshape(HID, NL),
            "W_reg": W_reg.reshape(HID, 1).astype(bfl),
            "idx": pre["idx_arrs"][c],
            "dstc": pre["dstc_arrs"][c],
            "iota": iota,
            "dinvp": pre["dinvp"][c],
            "dinvp2": pre["dinvp2"][c],
            "dinvb": pre["dinv_bc"][c],
            "bpool": pre["bpool"][c],
        })
    return in_maps


# expose pieces for test harness
def build_all(inputs):
    pre = _preprocess(np.asarray(inputs["edge_index"]), np.asarray(inputs["batch"]))
    nc = _build_program(pre["tmpl"])
    in_maps = _make_inputs(pre, np.asarray(inputs["x"], dtype=np.float32),
                           np.asarray(inputs["W_enc"], dtype=np.float32),
                           np.asarray(inputs["b_enc"], dtype=np.float32),
                           np.asarray(inputs["gcn_W"], dtype=np.float32),
                           np.asarray(inputs["gcn_b"], dtype=np.float32),
                           np.asarray(inputs["W_reg"], dtype=np.float32))
    return pre, nc, in_maps


# revision 3
# speedup vs baseline: 1.0541x; 1.0136x over previous
"""Trainium2 Bass kernel v2 for nn_GCNNet (3-layer GCN, 50k nodes, 800k edges,
HID=128, 64 graphs) sharded across 8 NeuronCores.

Changes vs v1 baseline:
- bf16-only node-feature table (256B rows instead of hi|lo 512B): halves
  gather bytes, scatter matmuls, and AllGather traffic. rel-err target is
  2e-2; bf16 quantization lands ~1e-3.
- dma_gather issued as prepare_only + trigger_dma on 2 SWDGE queues so the
  gpsimd engine no longer blocks for the DMA duration; transfers from
  consecutive calls overlap.
- S (segment-sum selection) matrices are expanded on-device from a per-edge
  dst-column byte via a broadcast is_equal, instead of streaming 16MB/layer
  of precomputed S from HBM.
- dinv_dst vector and pool matrix resident in SBUF; encoder inputs in bf16.
- dst-side norm folded: h is stored "raw" (pre dinv_dst scale); the scale is
  applied via dinv^2 at the next table build and via dinv at the final
  regression. Eviction = (psum * dinv_bcast) + bias, relu ... kept explicit
  (2 DVE ops) -- actually h stored raw means eviction skips the dinv mult.
"""
import os
import numpy as np
import ml_dtypes

# ---- problem constants (hardcoded; kernel.py must be self-contained) ----
N = 50000
E = 800000
D_IN = 100
HID = 128
NL = 3
NG = 64

NCORES = 8
P = 128
W_DST = 64             # dst columns per window
NWIN = 98              # windows per core
NPCS = NWIN * W_DST    # 6272 slots per core
NGRP = NPCS // P       # 49 groups of 128 slots
T_HALF = 5             # K-tiles per src-half per window
T_WIN = 2 * T_HALF     # 10 tiles per window
NTILES = NWIN * T_WIN  # 980 tiles per core
NROWS = NCORES * NPCS  # 50176 table rows
HALF_ROW = NROWS // 2  # 25088 = cores 0..3
CPW = 7                # windows per gather/scatter chunk
NCHUNK = NWIN // CPW   # 14 chunks
DELEM = HID            # 128 bf16 per table row = 256B

CALL_T = CPW * T_HALF           # tiles per gather call (35)
CALL_I = CALL_T * P             # idxs per call (4480)

_cache = {}


# ======================= host preprocessing =======================

def _snake(order, nbins):
    """Assign sorted items to bins in snake order; returns bin id per item."""
    n = len(order)
    assert n % nbins == 0
    rounds = n // nbins
    cols = np.tile(np.arange(nbins), (rounds, 1))
    cols[1::2] = cols[1::2][:, ::-1]
    bin_of = np.empty(n, dtype=np.int64)
    bin_of[order] = cols.ravel()
    return bin_of


def _preprocess(edge_index, batch):
    src = np.asarray(edge_index[0], dtype=np.int64)
    dst = np.asarray(edge_index[1], dtype=np.int64)
    batch = np.asarray(batch, dtype=np.int64)

    deg = (np.bincount(dst, minlength=N) + 1).astype(np.float32)
    dinv = (1.0 / np.sqrt(deg)).astype(np.float32)

    # ---- node -> core assignment, snake-balanced by in-degree(+1) ----
    w = deg.astype(np.int64)
    order = np.argsort(-w, kind="stable")
    node_core = _snake(order, NCORES).astype(np.int32)

    # half A = src owned by cores 0..3; self-loops are computed locally
    # (not gathered), so they do not contribute to edge counts.
    src_half = (node_core[src] >= 4).astype(np.int64)
    a_cnt = np.bincount(dst[src_half == 0], minlength=N)
    b_cnt = np.bincount(dst[src_half == 1], minlength=N)

    # ---- per-core window assignment (snake by total, repair half caps) ----
    CAP = T_HALF * P  # 640 per half
    node_slot = np.full(N, -1, dtype=np.int64)
    slot_node = np.full(NCORES * NPCS, -1, dtype=np.int64)
    wa_ranks = []
    wb_ranks = []
    tot = a_cnt + b_cnt
    LCAP = 4 * P  # light windows: 4 tiles per half
    for c in range(NCORES):
        nodes = np.nonzero(node_core == c)[0]
        npad = NPCS - len(nodes)
        # pad with fake node ids (-1) of zero weight
        ww = np.concatenate([tot[nodes], np.zeros(npad, dtype=np.int64)])
        ids = np.concatenate([nodes, np.full(npad, -1, dtype=np.int64)])
        order_c = np.argsort(-ww, kind="stable")
        # skewed first-fit: N_HEAVY windows cap 5 tiles/half, rest 4
        for N_HEAVY in (16, 24, 32, 48, NWIN):
            capA = np.full(NWIN, LCAP, dtype=np.int64)
            capB = np.full(NWIN, LCAP, dtype=np.int64)
            capA[:N_HEAVY] = CAP
            capB[:N_HEAVY] = CAP
            wa = np.zeros(NWIN, dtype=np.int64)
            wb = np.zeros(NWIN, dtype=np.int64)
            wc = np.zeros(NWIN, dtype=np.int64)
            win_of = np.empty(NPCS, dtype=np.int64)
            ok = True
            RES = 64
            for i in order_c:
                v = ids[i]
                av = a_cnt[v] if v >= 0 else 0
                bv = b_cnt[v] if v >= 0 else 0
                feas = np.nonzero((wc < W_DST) & (wa + av <= capA - RES)
                                  & (wb + bv <= capB - RES))[0]
                if len(feas) == 0:
                    feas = np.nonzero((wc < W_DST) & (wa + av <= capA)
                                      & (wb + bv <= capB))[0]
                if len(feas) == 0:
                    ok = False
                    break
                j = feas[np.argmax((wa[feas] + wb[feas]) * 100 - wc[feas])]
                win_of[i] = j
                wa[j] += av; wb[j] += bv; wc[j] += 1
            if ok:
                break
        else:
            raise RuntimeError("skew packing overflow")
        # relabel windows by decreasing load so window id == rank on every
        # core (SPMD-static variable-tile template)
        rank_of = np.empty(NWIN, dtype=np.int64)
        rank_of[np.argsort(-(wa + wb), kind="stable")] = np.arange(NWIN)
        win_of = rank_of[win_of]
        wa_r = np.zeros(NWIN, dtype=np.int64)
        wb_r = np.zeros(NWIN, dtype=np.int64)
        for i in range(NPCS):
            v = ids[i]
            if v >= 0:
                wa_r[win_of[i]] += a_cnt[v]
                wb_r[win_of[i]] += b_cnt[v]
        wa_ranks.append(wa_r)
        wb_ranks.append(wb_r)
        # slot within window by arrival order
        slot_in_win = np.zeros(NPCS, dtype=np.int64)
        cnt_w = np.zeros(NWIN, dtype=np.int64)
        for i in range(NPCS):
            slot_in_win[i] = cnt_w[win_of[i]]
            cnt_w[win_of[i]] += 1
        assert cnt_w.max() <= W_DST
        glob = c * NPCS + win_of * W_DST + slot_in_win
        real = ids >= 0
        node_slot[ids[real]] = glob[real]
        slot_node[glob[real]] = ids[real]

    perm = node_slot
    # per-window tile template (shared across cores)
    wa_ranks = np.stack(wa_ranks)
    wb_ranks = np.stack(wb_ranks)
    tA = np.maximum(np.ceil(wa_ranks.max(axis=0) / P).astype(np.int64), 1)
    tB = np.maximum(np.ceil(wb_ranks.max(axis=0) / P).astype(np.int64), 1)

    # ---- static chunk/call template from tile counts ----
    wic = [[wi for wi in range(NWIN) if wi % NCHUNK == ch] for ch in range(NCHUNK)]
    offA = np.zeros(NWIN, dtype=np.int64)  # tile offset of window within its A call
    offB = np.zeros(NWIN, dtype=np.int64)
    callA_T = np.zeros(NCHUNK, dtype=np.int64)
    callB_T = np.zeros(NCHUNK, dtype=np.int64)
    for ch in range(NCHUNK):
        a = b = 0
        for wi in wic[ch]:
            offA[wi] = a; a += tA[wi]
            offB[wi] = b; b += tB[wi]
        callA_T[ch] = a
        callB_T[ch] = b
    # idx stream element offset of each call (order: ch asc, half A then B)
    call_off = np.zeros((NCHUNK, 2), dtype=np.int64)
    tile_off = np.zeros((NCHUNK, 2), dtype=np.int64)  # tile-stream offsets
    acc_e = acc_t = 0
    for ch in range(NCHUNK):
        for h in (0, 1):
            call_off[ch, h] = acc_e
            tile_off[ch, h] = acc_t
            t = int(callA_T[ch] if h == 0 else callB_T[ch])
            acc_e += t * P
            acc_t += t
    NTILES_TOT = int(acc_t)
    tmpl = dict(tA=tA, tB=tB, wic=wic, offA=offA, offB=offB,
                callA_T=callA_T, callB_T=callB_T, call_off=call_off,
                tile_off=tile_off, ntiles=NTILES_TOT,
                mot=int(max(callA_T.max(), callB_T.max())),
                smax=int((callA_T + callB_T).max()))

    # ---- per-core edge slot assignment (no self-loops) ----
    all_src = src
    all_dst = dst
    e_row = perm[all_src]
    e_half = (e_row >= HALF_ROW).astype(np.int64)
    e_rel = e_row - e_half * HALF_ROW
    e_dslot = perm[all_dst]
    e_core = e_dslot // NPCS
    e_local = e_dslot % NPCS
    e_win = e_local // W_DST
    e_j = e_local % W_DST

    key = ((e_core * NWIN + e_win) * 2 + e_half)
    eorder = np.argsort(key, kind="stable")
    key_s = key[eorder]
    rel_s = e_rel[eorder].astype(np.int32)
    j_s = e_j[eorder].astype(np.int32)
    grp_start = np.searchsorted(key_s, np.arange(NCORES * NWIN * 2))
    grp_end = np.searchsorted(key_s, np.arange(NCORES * NWIN * 2), side="right")

    idx_arrs = []   # per core: int16 [P, NTILES_TOT*P//16] in call order
    dstc_arrs = []  # per core: bf16 [P, NTILES_TOT] dst column per slot (64=pad)
    for c in range(NCORES):
        idx_flat = np.zeros(NTILES_TOT * P, dtype=np.int32)
        dstc_flat = np.full(NTILES_TOT * P, W_DST, dtype=np.float32)
        for ch in range(NCHUNK):
            for h in (0, 1):
                offw = offA if h == 0 else offB
                tw = tA if h == 0 else tB
                for wi in wic[ch]:
                    g = (c * NWIN + wi) * 2 + h
                    lo, hi = grp_start[g], grp_end[g]
                    cnt = hi - lo
                    assert cnt <= tw[wi] * P, (c, ch, h, wi, cnt, tw[wi])
                    off = call_off[ch, h] + offw[wi] * P
                    idx_flat[off:off + cnt] = rel_s[lo:hi]
                    dstc_flat[off:off + cnt] = j_s[lo:hi]
        # idx wrap per call: idx i of call -> [i%16, i//16], replicated x8
        idx16 = np.zeros((P, NTILES_TOT * P // 16), dtype=np.int16)
        for ch in range(NCHUNK):
            for h in (0, 1):
                L = int((callA_T[ch] if h == 0 else callB_T[ch]) * P)
                e0 = int(call_off[ch, h])
                blk = idx_flat[e0:e0 + L].reshape(L // 16, 16).T.astype(np.int16)
                for k in range(8):
                    idx16[16 * k:16 * (k + 1), e0 // 16:(e0 + L) // 16] = blk
        idx_arrs.append(idx16)
        dstc_arrs.append(dstc_flat.reshape(NTILES_TOT, P).T
                         .astype(ml_dtypes.bfloat16).copy())

    # ---- per-core auxiliary arrays ----
    dinv_slot = np.zeros(NCORES * NPCS, dtype=np.float32)
    valid = slot_node >= 0
    dinv_slot[valid] = dinv[slot_node[valid]]

    dinvp = []      # [P, NGRP] f32 (power 1, layer-0 table build + final z)
    dinvp2 = []     # [P, NGRP] f32 (power 2, later table builds)
    dinv_bc = []    # [P, NPCS] bf16 replicated (dst-scale at eviction)
    bpool = []      # [P, NGRP*NG] f32
    for c in range(NCORES):
        ds = dinv_slot[c * NPCS:(c + 1) * NPCS]
        dp = ds.reshape(NGRP, P).T.copy()
        dinvp.append(dp)
        dinvp2.append((dp * dp).copy())
        dinv_bc.append(np.broadcast_to(ds.astype(ml_dtypes.bfloat16), (P, NPCS)).copy())
        sn = slot_node[c * NPCS:(c + 1) * NPCS]
        bp = np.zeros((P, NGRP * NG), dtype=np.float32)
        g_idx = np.arange(NPCS) // P
        p_idx = np.arange(NPCS) % P
        ok = sn >= 0
        bp[p_idx[ok], g_idx[ok] * NG + batch[sn[ok]]] = 1.0
        bpool.append(bp)

    cnt_g = np.bincount(batch, minlength=NG).astype(np.float32)
    return dict(perm=perm, slot_node=slot_node, dinv=dinv, cnt_g=cnt_g,
                idx_arrs=idx_arrs, dstc_arrs=dstc_arrs, dinvp=dinvp,
                dinvp2=dinvp2, dinv_bc=dinv_bc, bpool=bpool, tmpl=tmpl)


# ======================= bass program =======================

def _build_program(tmpl):
    import concourse.bass as bass
    import concourse.tile as tile
    from concourse import bacc, mybir
    from contextlib import ExitStack

    tA, tB, wic = tmpl["tA"], tmpl["tB"], tmpl["wic"]
    offA, offB = tmpl["offA"], tmpl["offB"]
    callA_T, callB_T = tmpl["callA_T"], tmpl["callB_T"]
    call_off, tile_off = tmpl["call_off"], tmpl["tile_off"]
    NTILES_TOT, MOT, SMAX = tmpl["ntiles"], tmpl["mot"], tmpl["smax"]

    f32 = mybir.dt.float32
    bf16 = mybir.dt.bfloat16
    i16 = mybir.dt.int16

    NQ = int(os.environ.get("GCN_NQ", "2"))
    nc = bacc.Bacc("TRN2", target_bir_lowering=False, debug=False,
                   num_devices=NCORES, enable_asserts=False,
                   num_swdge_queues=NQ)

    # ---- IO ----
    xT = nc.dram_tensor("xT", [D_IN, NPCS], bf16, kind="ExternalInput").ap()
    W_enc = nc.dram_tensor("W_enc", [D_IN, HID], bf16, kind="ExternalInput").ap()
    b_enc = nc.dram_tensor("b_enc", [HID, 1], f32, kind="ExternalInput").ap()
    gcn_W = nc.dram_tensor("gcn_W", [HID, NL * HID], bf16, kind="ExternalInput").ap()
    gcn_b = nc.dram_tensor("gcn_b", [HID, NL], f32, kind="ExternalInput").ap()
    W_reg = nc.dram_tensor("W_reg", [HID, 1], bf16, kind="ExternalInput").ap()
    idx_in = nc.dram_tensor("idx", [P, NTILES_TOT * P // 16], i16, kind="ExternalInput").ap()
    dstc_in = nc.dram_tensor("dstc", [P, NTILES_TOT], bf16, kind="ExternalInput").ap()
    iota_in = nc.dram_tensor("iota", [P, W_DST], bf16, kind="ExternalInput").ap()
    dinvp_in = nc.dram_tensor("dinvp", [P, NGRP], f32, kind="ExternalInput").ap()
    dinvp2_in = nc.dram_tensor("dinvp2", [P, NGRP], f32, kind="ExternalInput").ap()
    dinvb_in = nc.dram_tensor("dinvb", [P, NPCS], bf16, kind="ExternalInput").ap()
    bpool_in = nc.dram_tensor("bpool", [P, NGRP * NG], f32, kind="ExternalInput").ap()
    out_ext = nc.dram_tensor("pool_out", [NG, 1], f32, kind="ExternalOutput").ap()

    # ---- internal DRAM ----
    chunk_d = [nc.dram_tensor(f"chunk{i}", [NPCS, DELEM], bf16).ap()
               for i in range(NL)]
    table_d = [nc.dram_tensor(f"table{i}", [NROWS, DELEM], bf16,
                              addr_space="Shared").ap() for i in range(NL)]

    from concourse import library_config
    with tile.TileContext(nc) as tc, ExitStack() as ctx:
        pers = ctx.enter_context(tc.tile_pool(name="pers", bufs=1))
        msgs_p = ctx.enter_context(tc.tile_pool(name="msgs", bufs=4))
        s_p = ctx.enter_context(tc.tile_pool(name="sstream", bufs=2))
        stg_p = ctx.enter_context(tc.tile_pool(name="stg", bufs=3))
        ev_p = ctx.enter_context(tc.tile_pool(name="ev", bufs=3))
        ps_win = ctx.enter_context(tc.tile_pool(name="pswin", bufs=4, space="PSUM"))
        ps_tb = ctx.enter_context(tc.tile_pool(name="pstb", bufs=2, space="PSUM"))
        ps_misc = ctx.enter_context(tc.tile_pool(name="psmisc", bufs=1, space="PSUM"))

        # ---- resident tiles ----
        h_bufs = [pers.tile([P, NPCS], bf16, tag=f"h{i}", name=f"h{i}") for i in range(2)]
        hs_sb = pers.tile([P, NPCS], bf16, tag="hs")
        idx_sb = pers.tile([P, NTILES_TOT * P // 16], i16, tag="idx")
        dstc_sb = pers.tile([P, NTILES_TOT], bf16, tag="dstc")
        iota_sb = pers.tile([P, W_DST], bf16, tag="iota")
        bpool_sb = pers.tile([P, NGRP * NG], f32, tag="bpool")
        dinvp_sb = pers.tile([P, NGRP], f32, tag="dinvp")
        dinvp2_sb = pers.tile([P, NGRP], f32, tag="dinvp2")
        dinvb_sb = pers.tile([P, NPCS], bf16, tag="dinvb")
        wenc_sb = pers.tile([P, HID], bf16, tag="wenc")
        benc_sb = pers.tile([P, 1], f32, tag="benc")
        gcnw_sb = pers.tile([P, NL * HID], bf16, tag="gcnw")
        gcnb_sb = pers.tile([P, NL], f32, tag="gcnb")
        wreg_sb = pers.tile([P, 1], bf16, tag="wreg")
        zbuf = pers.tile([P, NGRP], f32, tag="zbuf")

        nc.gpsimd.load_library(library_config.mlp)
        nc.sync.dma_start(idx_sb[:], idx_in[:])
        nc.sync.dma_start(dstc_sb[:], dstc_in[:])
        nc.sync.dma_start(iota_sb[:], iota_in[:])
        nc.sync.dma_start(bpool_sb[:], bpool_in[:])
        nc.sync.dma_start(dinvp_sb[:], dinvp_in[:])
        nc.sync.dma_start(dinvp2_sb[:], dinvp2_in[:])
        nc.sync.dma_start(dinvb_sb[:], dinvb_in[:])
        nc.sync.dma_start(wenc_sb[:D_IN, :], W_enc[:])
        nc.sync.dma_start(benc_sb[:], b_enc[:])
        nc.sync.dma_start(gcnw_sb[:], gcn_W[:])
        nc.sync.dma_start(gcnb_sb[:], gcn_b[:])
        nc.sync.dma_start(wreg_sb[:], W_reg[:])

        # gather call plumbing: persistent round-robin msgs buffers (the tile
        # pool scheduler does not insert WAR edges for deferred prep-mode
        # gather writes), rotating sems (a fast ring on call N+1 must not
        # mask a slow ring on call N via a shared counter), explicit
        # consumer-side wait_ge on the PE queue.
        NBUF = NQ + 1  # in-flight gather buffers per half
        SEMS_PER_Q = 8
        sem_q = [[nc.alloc_semaphore(f"gq{q}_{i}") for i in range(SEMS_PER_Q)]
                 for q in range(NQ)]
        sem_ctr = [0] * NQ
        mbufs = [[pers.tile([P, MOT * DELEM], bf16, tag=f"mb{h}_{i}",
                            name=f"mb{h}_{i}") for i in range(NBUF)]
                 for h in range(2)]

        def gather_call(tbl_half_ap, e0, nidx, q, buf):
            ctr = sem_ctr[q]
            sem_ctr[q] += 1
            sq = sem_q[q][ctr % SEMS_PER_Q]
            tgt = 16 * (ctr // SEMS_PER_Q + 1)
            nt = nidx // P
            nc.gpsimd.dma_gather(
                out_ap=buf[:, 0:nt * DELEM].rearrange("p (k d) -> p k d", d=DELEM),
                in_ap=tbl_half_ap,
                idxs_ap=idx_sb[:, e0 // 16:(e0 + nidx) // 16],
                num_idxs=nidx,
                num_idxs_reg=nidx,
                elem_size=DELEM,
                single_packet=False,
                queue_num=q,
                prepare_only=True,
                sem=sq,
            )
            nc.gpsimd.trigger_dma(count=1, queue_num=q)
            return sq, tgt

        # ---- encoder: h0 = x @ W_enc + b_enc (as [HID, slots], bf16) ----
        h = h_bufs[0]
        ENC_N = 512
        for s0 in range(0, NPCS, ENC_N):
            n = min(ENC_N, NPCS - s0)
            xt = stg_p.tile([P, ENC_N], bf16, tag="xt")
            nc.sync.dma_start(xt[:D_IN, :n], xT[:, s0:s0 + n])
            psum = ps_tb.tile([P, ENC_N], f32, space="PSUM", tag="tb", name="encps")
            nc.tensor.matmul(psum[:, :n], lhsT=wenc_sb[:D_IN, :], rhs=xt[:D_IN, :n],
                             start=True, stop=True)
            nc.vector.tensor_scalar_add(h[:, s0:s0 + n], psum[:, :n], benc_sb[:, 0:1])

        # ---- GCN layers ----
        # h0 holds true h (encoder); later h holds "raw" h (pre dinv_dst
        # scale) and the dst scale is folded into the next build via dinv^2.
        def build_group(li2, g, hsrc):
            Wl2 = gcnw_sb[:, li2 * HID:(li2 + 1) * HID]
            pt = ps_tb.tile([P, HID], f32, space="PSUM", tag="tb")
            nc.tensor.matmul(pt[:], lhsT=hsrc[:, g * P:(g + 1) * P], rhs=Wl2,
                             start=True, stop=True)
            stg = stg_p.tile([P, DELEM], bf16, tag="stg")
            nc.vector.tensor_scalar_mul(stg[:], pt[:], dinvp_sb[:, g:g + 1])
            nc.sync.dma_start(chunk_d[li2][g * P:(g + 1) * P, :], stg[:])

        def all_gather(li2):
            nc.gpsimd.collective_compute(
                "AllGather", mybir.AluOpType.bypass,
                replica_groups=[list(range(NCORES))],
                ins=[chunk_d[li2][:]], outs=[table_d[li2][:]],
            )

        # groups of layer li+1 buildable after chunk ch of layer li
        ready_after = [[] for _ in range(NCHUNK)]
        for g in range(NGRP):
            ready_after[max((2 * g) % NCHUNK, (2 * g + 1) % NCHUNK)].append(g)

        # layer-0 table from encoder output
        for g in range(NGRP):
            build_group(0, g, h)
        all_gather(0)

        for li in range(NL):
            h_nxt = h_bufs[(li + 1) % 2]
            tbl = table_d[li]
            bl = gcnb_sb[:, li:li + 1]
            Wl = gcnw_sb[:, li * HID:(li + 1) * HID]

            # hs = h * dinv_dst (self-loop contributions, computed locally)
            nc.vector.tensor_tensor(out=hs_sb[:], in0=h[:], in1=dinvb_sb[:],
                                    op=mybir.AluOpType.mult)

            # gather + scatter chunks
            for ch in range(NCHUNK):
                cAT = int(callA_T[ch])
                cBT = int(callB_T[ch])
                mg = [None, None]
                waits = []
                for hh in (0, 1):
                    src_ap = tbl[0:HALF_ROW, :] if hh == 0 else tbl[HALF_ROW:NROWS, :]
                    mg[hh] = mbufs[hh][ch % NBUF]
                    nidx = (cAT if hh == 0 else cBT) * P
                    waits.append(gather_call(src_ap, int(call_off[ch, hh]), nidx,
                                             (ch * 2 + hh) % NQ, mg[hh]))

                # expand S tiles for this chunk from dst columns
                nst = cAT + cBT
                st = s_p.tile([P, SMAX * W_DST], bf16, tag="s")
                nc.vector.tensor_tensor(
                    out=st[:, 0:nst * W_DST].rearrange("p (t j) -> p t j", j=W_DST),
                    in0=iota_sb[:, 0:W_DST].unsqueeze(1)
                        .broadcast_to([P, nst, W_DST]),
                    in1=dstc_sb[:, int(tile_off[ch, 0]):int(tile_off[ch, 0]) + nst]
                        .unsqueeze(2).broadcast_to([P, nst, W_DST]),
                    op=mybir.AluOpType.is_equal)

                for sq, tgt in waits:
                    nc.tensor.wait_ge(sq, tgt)
                for wi in wic[ch]:
                    pw = ps_win.tile([P, W_DST], f32, space="PSUM", tag="win")
                    nc.tensor.matmul(
                        pw[:], lhsT=Wl,
                        rhs=hs_sb[:, wi * W_DST:(wi + 1) * W_DST],
                        start=True, stop=False)
                    for hh in (0, 1):
                        tw = int((tA if hh == 0 else tB)[wi])
                        ow = int((offA if hh == 0 else offB)[wi])
                        sbase = (0 if hh == 0 else cAT) + ow
                        for kk in range(tw):
                            tloc = ow + kk
                            srow = (sbase + kk) * W_DST
                            last = (hh == 1 and kk == tw - 1)
                            nc.tensor.matmul(
                                pw[:], lhsT=mg[hh][:, tloc * DELEM:(tloc + 1) * DELEM],
                                rhs=st[:, srow:srow + W_DST],
                                start=False, stop=last)
                    sc = ev_p.tile([P, W_DST], f32, tag="sc")
                    nc.vector.tensor_tensor(out=sc[:], in0=pw[:],
                                            in1=dinvb_sb[:, wi * W_DST:(wi + 1) * W_DST],
                                            op=mybir.AluOpType.mult)
                    nc.vector.tensor_scalar(
                        out=h_nxt[:, wi * W_DST:(wi + 1) * W_DST], in0=sc[:],
                        scalar1=bl, scalar2=0.0,
                        op0=mybir.AluOpType.add, op1=mybir.AluOpType.max)
                if li + 1 < NL:
                    for g in ready_after[ch]:
                        build_group(li + 1, g, h_nxt)
            if li + 1 < NL:
                all_gather(li + 1)
            h = h_nxt

        # ---- regression + pool ----
        for g in range(NGRP):
            pz = ps_misc.tile([P, 1], f32, space="PSUM", tag="z", bufs=1)
            nc.tensor.matmul(pz[:], lhsT=h[:, g * P:(g + 1) * P], rhs=wreg_sb[:],
                             start=True, stop=True)
            nc.vector.tensor_copy(zbuf[:, g:g + 1], pz[:])
        pp = ps_misc.tile([NG, 1], f32, space="PSUM", tag="pool", bufs=1)
        for g in range(NGRP):
            nc.tensor.matmul(pp[:], lhsT=bpool_sb[:, g * NG:(g + 1) * NG],
                             rhs=zbuf[:, g:g + 1],
                             start=(g == 0), stop=(g == NGRP - 1))
        outt = ev_p.tile([NG, 1], f32, tag="out")
        nc.vector.tensor_copy(outt[:], pp[:])
        nc.sync.dma_start(out_ext[:], outt[:])

    nc.compile()
    return nc


# ======================= entry point =======================

def kernel(x, edge_index, batch, W_enc, b_enc, gcn_W, gcn_b, W_reg, b_reg):
    x = np.asarray(x, dtype=np.float32)
    edge_index = np.asarray(edge_index)
    batch = np.asarray(batch)
    W_enc = np.asarray(W_enc, dtype=np.float32)
    b_enc = np.asarray(b_enc, dtype=np.float32)
    gcn_W = np.asarray(gcn_W, dtype=np.float32)
    gcn_b = np.asarray(gcn_b, dtype=np.float32)
    W_reg = np.asarray(W_reg, dtype=np.float32)
    b_reg = np.asarray(b_reg, dtype=np.float32)

    key = (edge_index.tobytes(), batch.tobytes())
    pk = hash(key)
    if pk not in _cache:
        pre = _preprocess(edge_index, batch)
        nc = _build_program(pre["tmpl"])
        _cache.clear()
        _cache[pk] = (pre, nc)
    pre, nc = _cache[pk]

    in_maps = _make_inputs(pre, x, W_enc, b_enc, gcn_W, gcn_b, W_reg)

    from concourse.bass_utils import run_bass_kernel_spmd
    res = run_bass_kernel_spmd(nc, in_maps, core_ids=list(range(NCORES)),
                               trace=bool(int(os.environ.get("GCN_TRACE", "0"))))
    if res.exec_time_ns is not None:
        print(f"HW exec time: {res.exec_time_ns} ns", flush=True)

    pool = np.zeros((NG, 1), dtype=np.float32)
    for c in range(NCORES):
        pool += res.results[c]["pool_out"]
    out = pool / np.maximum(pre["cnt_g"], 1.0)[:, None] + b_reg
    return out.astype(np.float32)


def _make_inputs(pre, x, W_enc, b_enc, gcn_W, gcn_b, W_reg):
    bfl = ml_dtypes.bfloat16
    in_maps = []
    slot_node = pre["slot_node"]
    iota = np.broadcast_to(np.arange(W_DST, dtype=np.float32), (P, W_DST))
    iota = iota.astype(bfl)
    for c in range(NCORES):
        sn = slot_node[c * NPCS:(c + 1) * NPCS]
        xTc = np.zeros((D_IN, NPCS), dtype=bfl)
        valid = sn >= 0
        xTc[:, valid] = x[sn[valid]].T.astype(bfl)
        in_maps.append({
            "xT": xTc,
            "W_enc": W_enc.astype(bfl),
            "b_enc": b_enc.reshape(HID, 1),
            "gcn_W": np.concatenate([gcn_W[l] for l in range(NL)], axis=1).astype(bfl),
            "gcn_b": gcn_b.T.copy().reshape(HID, NL),
            "W_reg": W_reg.reshape(HID, 1).astype(bfl),
            "idx": pre["idx_arrs"][c],
            "dstc": pre["dstc_arrs"][c],
            "iota": iota,
            "dinvp": pre["dinvp"][c],
            "dinvp2": pre["dinvp2"][c],
            "dinvb": pre["dinv_bc"][c],
            "bpool": pre["bpool"][c],
        })
    return in_maps


# expose pieces for test harness
def build_all(inputs):
    pre = _preprocess(np.asarray(inputs["edge_index"]), np.asarray(inputs["batch"]))
    nc = _build_program(pre["tmpl"])
    in_maps = _make_inputs(pre, np.asarray(inputs["x"], dtype=np.float32),
                           np.asarray(inputs["W_enc"], dtype=np.float32),
                           np.asarray(inputs["b_enc"], dtype=np.float32),
                           np.asarray(inputs["gcn_W"], dtype=np.float32),
                           np.asarray(inputs["gcn_b"], dtype=np.float32),
                           np.asarray(inputs["W_reg"], dtype=np.float32))
    return pre, nc, in_maps
